# revision 1
# baseline (speedup 1.0000x reference)
"""Trainium2 Bass kernel for nn_CHILDREN_TENSOR (gnn_message_passing).

bf16 gather + cast-on-store design.

dma_gather pulls bf16 rows (256 B) from HBM into SBUF across 4 SWDGE queues
(the gather is queue-descriptor-drain bound at ~8.5 ns/desc/queue, so 4
queues ~ 277 us/iter). The store is a gpsimd-initiated dma_start that
upcasts bf16 -> f32 in the DMA itself (only gpsimd DMAs may cast), writing
f32 to HBM. No Act/DVE conversion stage, no f32 staging buffer.

Pool issues gathers for group gs, then the cast-store for group gs-LAG, so
descriptor generation never blocks on the drain of the group it just issued.
"""

import sys

for _p in ("/opt/trn_rl_repo",):
    if _p not in sys.path:
        sys.path.insert(0, _p)

from contextlib import ExitStack

import numpy as np

import concourse.bacc as bacc
import concourse.mybir as mybir
from concourse.bass_utils import run_bass_kernel_spmd

B, N, C, F = 16, 2048, 32, 128
N_CORES = 8
B_PER_CORE = B // N_CORES            # 2
ROWS_PER_BATCH = N * C               # 65536
ROWS_PER_CORE = B_PER_CORE * ROWS_PER_BATCH          # 131072

GATHER_ROWS = 1024                   # rows per dma_gather
G_SUB = GATHER_ROWS // 128           # 8 free-dim blocks per gather
IDX_COLS = GATHER_ROWS // 16         # 64 idx columns per gather

GROUP_ROWS = 4096                    # rows per store
G = GROUP_ROWS // 128                # 64 free-dim blocks per group buffer
GATHERS_PER_GROUP = GROUP_ROWS // GATHER_ROWS        # 8
N_GROUPS = ROWS_PER_CORE // GROUP_ROWS               # 16 per iteration
N_GATHERS = N_GROUPS * GATHERS_PER_GROUP             # 128 per iteration

NSEMS = 16
NBUFS = 12                           # bf16 group buffers (8 KiB/part each)
LAG = 4                              # store trails gather issue by LAG groups


def build_nc(repeat=1, timing_build=False, mode="full", nq=4, nbufs=None, lag=None, mdl=None, qrot=0, qstore=False, nsems=None):
    NSEMS = globals()['NSEMS'] if nsems is None else nsems
    nbufs = NBUFS if nbufs is None else nbufs
    lag = LAG if lag is None else lag
    nc = bacc.Bacc(
        "TRN2", debug=False, target_bir_lowering=False,
        num_swdge_queues=nq,
    )

    nodes = nc.dram_tensor(
        "nodes", [B_PER_CORE, N, F], mybir.dt.bfloat16,
        kind="Internal" if timing_build else "ExternalInput",
    )
    idxs = nc.dram_tensor(
        "idxs", [128, N_GATHERS * IDX_COLS], mybir.dt.int16, kind="ExternalInput"
    )
    out = nc.dram_tensor(
        "out", [ROWS_PER_CORE, F], mybir.dt.float32,
        kind="Internal" if timing_build else "ExternalOutput",
    )
    tok = (
        nc.dram_tensor("tok", [1, F], mybir.dt.float32, kind="ExternalOutput")
        if timing_build else None
    )

    do_gather = mode in ("full", "gather")
    do_store = mode in ("full",)

    with (
        nc.sbuf_tensor(
            "idx_sb", [128, N_GATHERS * IDX_COLS], mybir.dt.int16
        ) as idx_sb,
        nc.sbuf_tensor("gbuf", [128, nbufs, G, F], mybir.dt.bfloat16) as gbuf,
        nc.semaphore("load_sem") as load_sem,
        nc.semaphore("done_sem") as done_sem,
        ExitStack() as stack,
        nc.Block() as block,
    ):
        g_sems = [
            stack.enter_context(nc.semaphore(f"g_sem{i}")) for i in range(NSEMS)
        ]
        st_sems = [
            stack.enter_context(nc.semaphore(f"st_sem{i}")) for i in range(NSEMS)
        ]
        total_groups = N_GROUPS * repeat

        def g_target(gs):
            return 16 * GATHERS_PER_GROUP * (gs // NSEMS + 1)

        st_per_group = GATHERS_PER_GROUP if qstore else 1

        def st_target(gs):
            return 16 * st_per_group * (gs // NSEMS + 1)

        out_v = out.rearrange("(s p gf) f -> s p (gf f)", p=128, gf=G)
        gb_v = gbuf.rearrange("p n g f -> p n (g f)")

        @block.gpsimd
        def _(gpsimd):
            gpsimd.dma_start(idx_sb[:], idxs[:]).then_inc(load_sem, 16)
            gpsimd.wait_ge(load_sem, 16)

            def issue_store(gs):
                s = gs % N_GROUPS
                if qstore:
                    # quarter-stores: quarter j waits only its own gather's
                    # per-slot sem, avoiding the max-of-4-queues barrier.
                    for j in range(GATHERS_PER_GROUP):
                        gi = gs * GATHERS_PER_GROUP + j
                        gpsimd.wait_ge(
                            g_sems[gi % NSEMS], 16 * (gi // NSEMS + 1)
                        )
                        w = G_SUB * F
                        gpsimd.dma_start(
                            out_v[s, :, j * w:(j + 1) * w],
                            gb_v[:, gs % nbufs, j * w:(j + 1) * w],
                        ).then_inc(st_sems[gs % NSEMS], 16)
                else:
                    gpsimd.wait_ge(g_sems[gs % NSEMS], g_target(gs))
                    gpsimd.dma_start(
                        out_v[s], gb_v[:, gs % nbufs], max_dma_last_dim=mdl
                    ).then_inc(st_sems[gs % NSEMS], 16)

            if do_gather:
                for gs in range(total_groups):
                    s = gs % N_GROUPS
                    b = s // (N_GROUPS // B_PER_CORE)
                    if gs >= nbufs:
                        dep = st_sems if do_store else g_sems
                        tgt = (st_target if do_store else g_target)(gs - nbufs)
                        gpsimd.wait_ge(dep[(gs - nbufs) % NSEMS], tgt)
                    for j in range(GATHERS_PER_GROUP):
                        gi = s * GATHERS_PER_GROUP + j
                        col = gi * IDX_COLS
                        gpsimd.dma_gather(
                            gbuf[:, gs % nbufs, j * G_SUB:(j + 1) * G_SUB],
                            nodes[b],
                            idx_sb[:, col:col + IDX_COLS],
                            GATHER_ROWS,
                            GATHER_ROWS,
                            F,
                            queue_num=(gs * GATHERS_PER_GROUP + j + qrot * gs) % nq,
                        ).then_inc(
                            g_sems[
                                (gs * GATHERS_PER_GROUP + j) % NSEMS
                                if qstore else gs % NSEMS
                            ],
                            16,
                        )
                    if do_store and gs >= lag:
                        issue_store(gs - lag)
                if do_store:
                    for gs in range(total_groups - lag, total_groups):
                        issue_store(gs)
            gpsimd.sem_inc(done_sem, 1)

        @block.sync
        def _(sync):
            if do_store:
                for i in range(NSEMS):
                    sync.wait_ge(
                        st_sems[i],
                        16 * st_per_group * (total_groups // NSEMS),
                    )
            elif do_gather:
                for i in range(NSEMS):
                    sync.wait_ge(
                        g_sems[i],
                        16 * GATHERS_PER_GROUP * (total_groups // NSEMS),
                    )
            sync.wait_ge(done_sem, 1)
            if tok is not None:
                sync.dma_start(
                    tok[:], gb_v[:1, 0, :256].bitcast(mybir.dt.float32)
                ).then_inc(load_sem, 16)
                sync.wait_ge(load_sem, 32)

    nc.compile()
    return nc


def make_in_maps(nodes, children):
    """Identical index preprocessing to the f32 baseline; nodes -> bf16."""
    nodes_z = np.ascontiguousarray(np.asarray(nodes), dtype=np.float32).copy()
    nodes_z[:, 0, :] = 0.0
    nodes16 = nodes_z.astype(mybir.dt.np(mybir.dt.bfloat16))
    ch = np.ascontiguousarray(np.asarray(children)).astype(np.int16)

    in_maps = []
    for core in range(N_CORES):
        nb = nodes16[core * B_PER_CORE:(core + 1) * B_PER_CORE]
        cb = ch[core * B_PER_CORE:(core + 1) * B_PER_CORE].reshape(
            ROWS_PER_CORE
        )
        r = cb.reshape(N_GROUPS, 128, GATHERS_PER_GROUP, G_SUB)
        r = r.transpose(0, 2, 3, 1).reshape(N_GATHERS, GATHER_ROWS)
        w = r.reshape(N_GATHERS, IDX_COLS, 16)
        w = w.transpose(2, 0, 1).reshape(16, N_GATHERS * IDX_COLS)
        idx_t = np.tile(w, (8, 1)).astype(np.int16)
        in_maps.append({"nodes": np.ascontiguousarray(nb), "idxs": idx_t})
    return in_maps


_NC_CACHE = None


def kernel(nodes, children, feature_size=None):
    global _NC_CACHE
    if _NC_CACHE is None:
        _NC_CACHE = build_nc()
    nc = _NC_CACHE

    in_maps = make_in_maps(nodes, children)
    res = run_bass_kernel_spmd(nc, in_maps, list(range(N_CORES))).results

    out = np.empty((B, N, C, F), np.float32)
    for core in range(N_CORES):
        out[core * B_PER_CORE:(core + 1) * B_PER_CORE] = (
            res[core]["out"].reshape(B_PER_CORE, N, C, F)
        )
    return out



# revision 2
# speedup vs baseline: 1.0276x; 1.0276x over previous
"""Trainium2 Bass kernel for nn_CHILDREN_TENSOR (gnn_message_passing).

v2: bf16 SWDGE gather + ACT-engine upcast + HWDGE f32 stores.

The gather (131072 rows/core of 256 B from HBM) is descriptor-drain bound
on the 4 SWDGE queues (~1.5-1.9 ns/row). v1 also pushed the bf16->f32
cast-stores through the same SWDGE queues, adding ~40-80 us of
contention. v2 keeps SWDGE purely for gathers: the Activation engine
upcasts each 4096-row group bf16->f32 in SBUF (~3.4 us/group, fully
hidden), and the sync (SP) engine issues fat f32 stores through HWDGE,
which run at >600 GB/s and overlap with the gather drain.

Pipeline per group gs: gpsimd 4x dma_gather -> g_sem | ACT copy
gbuf->fbuf -> c_sem | SP dma_start fbuf->out -> st_sem. gbuf slots
recycle on c_sem (cast done), fbuf slots on st_sem (store done).
"""

import sys

for _p in ("/opt/trn_rl_repo",):
    if _p not in sys.path:
        sys.path.insert(0, _p)

from contextlib import ExitStack

import numpy as np

import concourse.bacc as bacc
import concourse.mybir as mybir
from concourse.bass_utils import run_bass_kernel_spmd

B, N, C, F = 16, 2048, 32, 128
N_CORES = 8
B_PER_CORE = B // N_CORES            # 2
ROWS_PER_BATCH = N * C               # 65536
ROWS_PER_CORE = B_PER_CORE * ROWS_PER_BATCH          # 131072

GATHER_ROWS = 1024                   # rows per dma_gather (ucode max)
G_SUB = GATHER_ROWS // 128           # 8 free-dim blocks per gather
IDX_COLS = GATHER_ROWS // 16         # 64 idx columns per gather

GROUP_ROWS = 4096                    # rows per cast/store group
G = GROUP_ROWS // 128                # 32 free-dim blocks per group buffer
GATHERS_PER_GROUP = GROUP_ROWS // GATHER_ROWS        # 4
N_GROUPS = ROWS_PER_CORE // GROUP_ROWS               # 32 per iteration
N_GATHERS = N_GROUPS * GATHERS_PER_GROUP             # 128 per iteration

NSEMS = 16
NB = 8                               # bf16 gather group buffers (8 KiB/part)
NF = 5                               # f32 staging buffers (16 KiB/part)


def build_nc(repeat=1, timing_build=False, mode="full", nq=4, nb=None,
             nf=None, sp=False):
    nb = NB if nb is None else nb
    nf = NF if nf is None else nf
    nc = bacc.Bacc(
        "TRN2", debug=False, target_bir_lowering=False,
        num_swdge_queues=nq,
    )

    nodes = nc.dram_tensor(
        "nodes", [B_PER_CORE, N, F], mybir.dt.bfloat16,
        kind="Internal" if timing_build else "ExternalInput",
    )
    idxs = nc.dram_tensor(
        "idxs", [128, N_GATHERS * IDX_COLS], mybir.dt.int16,
        kind="ExternalInput",
    )
    out = nc.dram_tensor(
        "out", [ROWS_PER_CORE, F], mybir.dt.float32,
        kind="Internal" if timing_build else "ExternalOutput",
    )
    tok = (
        nc.dram_tensor("tok", [1, F], mybir.dt.float32, kind="ExternalOutput")
        if timing_build else None
    )

    do_cast = mode in ("full", "cast")
    do_store = mode in ("full",)

    with (
        nc.sbuf_tensor(
            "idx_sb", [128, N_GATHERS * IDX_COLS], mybir.dt.int16
        ) as idx_sb,
        nc.sbuf_tensor("gbuf", [128, nb, G, F], mybir.dt.bfloat16) as gbuf,
        nc.sbuf_tensor("fbuf", [128, nf, G, F], mybir.dt.float32) as fbuf,
        nc.semaphore("load_sem") as load_sem,
        nc.semaphore("done_sem") as done_sem,
        ExitStack() as stack,
        nc.Block() as block,
    ):
        g_sems = [
            stack.enter_context(nc.semaphore(f"g_sem{i}")) for i in range(NSEMS)
        ]
        c_sems = [
            stack.enter_context(nc.semaphore(f"c_sem{i}")) for i in range(NSEMS)
        ]
        st_sems = [
            stack.enter_context(nc.semaphore(f"st_sem{i}")) for i in range(NSEMS)
        ]
        total_groups = N_GROUPS * repeat

        def g_target(gs):
            return 16 * GATHERS_PER_GROUP * (gs // NSEMS + 1)

        def c_target(gs):
            return gs // NSEMS + 1

        def st_target(gs):
            return 16 * (gs // NSEMS + 1)

        out_v = out.rearrange("(s p gf) f -> s p (gf f)", p=128, gf=G)
        gb_v = gbuf.rearrange("p n g f -> p n (g f)")
        fb_v = fbuf.rearrange("p n g f -> p n (g f)")

        @block.gpsimd
        def _(gpsimd):
            gpsimd.dma_start(idx_sb[:], idxs[:]).then_inc(load_sem, 16)
            gpsimd.wait_ge(load_sem, 16)
            for gs in range(total_groups):
                s = gs % N_GROUPS
                b = s // (N_GROUPS // B_PER_CORE)
                if gs >= nb and do_cast:
                    gpsimd.wait_ge(
                        c_sems[(gs - nb) % NSEMS], c_target(gs - nb)
                    )
                for j in range(GATHERS_PER_GROUP):
                    gi = s * GATHERS_PER_GROUP + j
                    col = gi * IDX_COLS
                    gpsimd.dma_gather(
                        gbuf[:, gs % nb, j * G_SUB:(j + 1) * G_SUB],
                        nodes[b],
                        idx_sb[:, col:col + IDX_COLS],
                        GATHER_ROWS,
                        GATHER_ROWS,
                        F,
                        single_packet=sp,
                        queue_num=(gs * GATHERS_PER_GROUP + j) % nq,
                    ).then_inc(g_sems[gs % NSEMS], 16)
            gpsimd.sem_inc(done_sem, 1)

        if do_cast:
            @block.scalar
            def _(scalar):
                for gs in range(total_groups):
                    scalar.wait_ge(g_sems[gs % NSEMS], g_target(gs))
                    if do_store and gs >= nf:
                        scalar.wait_ge(
                            st_sems[(gs - nf) % NSEMS], st_target(gs - nf)
                        )
                    scalar.copy(
                        fb_v[:, gs % nf], gb_v[:, gs % nb]
                    ).then_inc(c_sems[gs % NSEMS], 1)

        @block.sync
        def _(sync):
            if do_store:
                for gs in range(total_groups):
                    s = gs % N_GROUPS
                    sync.wait_ge(c_sems[gs % NSEMS], c_target(gs))
                    sync.dma_start(
                        out_v[s], fb_v[:, gs % nf]
                    ).then_inc(st_sems[gs % NSEMS], 16)
                for i in range(NSEMS):
                    sync.wait_ge(st_sems[i], 16 * (total_groups // NSEMS))
            elif do_cast:
                for i in range(NSEMS):
                    sync.wait_ge(c_sems[i], total_groups // NSEMS)
            else:
                for i in range(NSEMS):
                    sync.wait_ge(
                        g_sems[i],
                        16 * GATHERS_PER_GROUP * (total_groups // NSEMS),
                    )
            sync.wait_ge(done_sem, 1)
            if tok is not None:
                sync.dma_start(
                    tok[:], fb_v[:1, 0, :F] if do_cast
                    else gb_v[:1, 0, :256].bitcast(mybir.dt.float32)
                ).then_inc(load_sem, 16)
                sync.wait_ge(load_sem, 32)

    nc.compile()
    return nc


def make_in_maps(nodes, children):
    """Identical index preprocessing to v1; nodes -> bf16."""
    nodes_z = np.ascontiguousarray(np.asarray(nodes), dtype=np.float32).copy()
    nodes_z[:, 0, :] = 0.0
    nodes16 = nodes_z.astype(mybir.dt.np(mybir.dt.bfloat16))
    ch = np.ascontiguousarray(np.asarray(children)).astype(np.int16)

    in_maps = []
    for core in range(N_CORES):
        nb_ = nodes16[core * B_PER_CORE:(core + 1) * B_PER_CORE]
        cb = ch[core * B_PER_CORE:(core + 1) * B_PER_CORE].reshape(
            ROWS_PER_CORE
        )
        r = cb.reshape(N_GROUPS, 128, GATHERS_PER_GROUP, G_SUB)
        r = r.transpose(0, 2, 3, 1).reshape(N_GATHERS, GATHER_ROWS)
        w = r.reshape(N_GATHERS, IDX_COLS, 16)
        w = w.transpose(2, 0, 1).reshape(16, N_GATHERS * IDX_COLS)
        idx_t = np.tile(w, (8, 1)).astype(np.int16)
        in_maps.append({"nodes": np.ascontiguousarray(nb_), "idxs": idx_t})
    return in_maps


_NC_CACHE = None


def kernel(nodes, children, feature_size=None):
    global _NC_CACHE
    if _NC_CACHE is None:
        _NC_CACHE = build_nc()
    nc = _NC_CACHE

    in_maps = make_in_maps(nodes, children)
    res = run_bass_kernel_spmd(nc, in_maps, list(range(N_CORES))).results

    out = np.empty((B, N, C, F), np.float32)
    for core in range(N_CORES):
        out[core * B_PER_CORE:(core + 1) * B_PER_CORE] = (
            res[core]["out"].reshape(B_PER_CORE, N, C, F)
        )
    return out
